# revision 17
# baseline (speedup 1.0000x reference)
"""Trainium2 Bass kernel for nn_AttentionE (bf16 I/O pipeline).

Computes, per sample i:
    s_i   = sum(d_i)                       # d: (N, 6)
    z_ic  = W * s_i * e_ic + b_c           # e: (N, 5), W scalar, b: (5,)
    a_ic  = exp(tanh(z_ic))
    out_ic = e_ic * a_ic / sum_c(a_ic)     # (eps=1e-7 in ref; negligible)

Sharding: data-parallel over the sample axis across 8 NeuronCores.

This revision moves the HBM streams to bf16 (d, e inputs and the out
output), halving DMA traffic vs the f32 baseline: 64 B/sample ->
32 B/sample, memory roofline ~110us -> ~55us per pass. The pipeline
keeps f32 only where it matters (softmax denominator + reciprocal).
Host-side casts f32->bf16 (inputs) and bf16->f32 (output) keep the
kernel() contract at f32; rel-err of the bf16 path is ~1e-2 worst case
(gate 2e-2).

DVE bf16 notes (TRN2): packed 2-byte tensor_tensor runs at 2x; broadcast
APs and tensor_reduce stay 1x. So sum-of-6 is done as a packed pairwise
add (d[:, :, 0:3] + d[:, :, 3:6], 2x) + reduce3, and a*e is a packed 2x
multiply. The two broadcast multiplies (z = s*e, out = w*r) and the
reductions are the 1x DVE poles; engine assignment is env-tunable to
balance DVE against the GpSimd(Pool) engine on real HW.
"""

import sys

import numpy as np

_REPO = "/opt/trn_rl_repo"
if _REPO not in sys.path:
    sys.path.insert(0, _REPO)

from contextlib import ExitStack, nullcontext

import concourse.bacc as bacc
import concourse.bass as bass
import concourse.tile as tile
from concourse import mybir

N_CORES = 8
N_FULL = 4194304
P = 128  # SBUF partitions

import os as _os

# Tunables (env-overridable for bench sweeps)
M = int(_os.environ.get("K_M", "512"))  # samples per partition per tile
BUFS = int(_os.environ.get("K_BUFS", "4"))

# Engine assignment: "vector" or "gpsimd"
H3_ENGINE = _os.environ.get("K_H3_ENGINE", "vector")   # packed pairwise add (2x on DVE)
Z_ENGINE = _os.environ.get("K_Z_ENGINE", "gpsimd")     # z = s_b * e (broadcast, 1x)
W_ENGINE = _os.environ.get("K_W_ENGINE", "vector")     # w = a * e (packed, 2x on DVE)
OUT_ENGINE = _os.environ.get("K_OUT_ENGINE", "gpsimd") # out = w * r_b (broadcast, 1x)
# Number of out-stage components (0..5) computed on DVE instead of OUT_ENGINE.
OUT_SPLIT_K = int(_os.environ.get("K_OUT_SPLIT_K", "0"))
# sum6 mode: "h3" = packed pairwise add of the two 3-comp halves (2x on DVE),
# then SUM3 over the 3 partials; "reduce6" = plain tensor_reduce
SUM6 = _os.environ.get("K_SUM6", "h3")
# 3-way sum tail: "stt" = two strided adds (scalar_tensor_tensor + add,
# 2m elems); "reduce" = tensor_reduce over [m,3] (3m elems, 1 inst)
SUM3 = _os.environ.get("K_SUM3", "stt")
# reduce5 mode: "plain" = tensor_reduce (f16 in, f32 out, 5m elems);
# "tree" = packed h2 pairwise (2x, 2m) + strided stt-add chain (2m)
SUM5 = _os.environ.get("K_SUM5", "tree")
# Ramp-up/down: split the first/last tile into this many sub-tiles.
RAMP = int(_os.environ.get("K_RAMP", "2"))
RAMP_TAIL = int(_os.environ.get("K_RAMP_TAIL", "2"))
# Engine ring that issues the out DMA / the e-input DMA ("sync" = SP HWDGE
# ring, "scalar" = ACT HWDGE ring, "gpsimd" = SWDGE). ACT is nearly saturated
# by tanh/exp, so parking DMAs there stalls its sequencer (CoreSim): all-sync
# modeled 60.3us vs out-on-scalar 73.7us.
OUT_DMA = _os.environ.get("K_OUT_DMA", "sync")
E_DMA = _os.environ.get("K_E_DMA", "sync")
# Software-pipelined emission: emit front(t) [sum6, zmul, tanh, exp] before
# back(t-1) [sum5, recip, wmul, outmul, store] so engines (which execute in
# program order) overlap across tiles instead of stalling on the ACT
# round-trip of their own tile.
SW_PIPE = bool(int(_os.environ.get("K_SW_PIPE", "1")))
SW_DEPTH = int(_os.environ.get("K_SW_DEPTH", "1"))

# test.py can flip this to get profile/exec-time back
TRACE = False
LAST = {}

# Diagnostic modes for decomposition benches (never used by kernel()):
SKIP_COMPUTE = bool(int(_os.environ.get("K_SKIP_COMPUTE", "0")))
SKIP_DMA = bool(int(_os.environ.get("K_SKIP_DMA", "0")))


def build_bass(
    W: float,
    bvals,
    S: int,
    m: int = M,
    bufs: int = BUFS,
    repeats: int = 1,
    passes: int = 1,
):
    """Build the single-core SPMD program: d[S,6], e[S,5] bf16 -> out[S,5] bf16.

    repeats>1 wraps the whole tile loop in a hardware For_i so test.py can
    measure steady-state device time via the wall-clock slope over R.
    """
    assert S % (P * m) == 0, (S, P, m)
    T = S // (P * m)
    f32 = mybir.dt.float32
    f16 = mybir.dt.float16
    bf16 = mybir.dt.bfloat16
    mult = mybir.AluOpType.mult
    add = mybir.AluOpType.add
    X = mybir.AxisListType.X
    ACT = mybir.ActivationFunctionType

    nc = bacc.Bacc("TRN2", debug=False, num_devices=N_CORES)

    # Register the bias values as const APs so activation(bias=<float>) works.
    for i, v in enumerate(dict.fromkeys(float(x) for x in bvals)):
        t_c = nc.alloc_sbuf_tensor(f"const-bias-{i}", [P, 1], f32)
        nc.gpsimd.memset(t_c.ap(), v)
        nc.const_aps.aps[(f32, v)] = t_c.ap()
    nc.all_engine_barrier()

    d_ap = nc.dram_tensor("d", [S, 6], f16, kind="ExternalInput").ap()
    e_ap = nc.dram_tensor("e", [S, 5], f16, kind="ExternalInput").ap()
    o_ap = nc.dram_tensor("out", [S, 5], bf16, kind="ExternalOutput").ap()

    # [T, P, m*c] views; per partition the data is one contiguous DRAM run.
    d_v = d_ap.rearrange("(t p m) c -> t p (m c)", t=T, p=P, m=m)
    e_v = e_ap.rearrange("(t p m) c -> t p (m c)", t=T, p=P, m=m)
    o_v = o_ap.rearrange("(t p m) c -> t p (m c)", t=T, p=P, m=m)

    eng = {"vector": nc.vector, "gpsimd": nc.gpsimd}
    h3_eng = eng[H3_ENGINE]
    z_eng = eng[Z_ENGINE]
    w_eng = eng[W_ENGINE]
    out_eng = eng[OUT_ENGINE]
    dma_rings = {
        "sync": nc.sync,
        "scalar": nc.scalar,
        "tensor": nc.tensor,
        "gpsimd": nc.gpsimd,
    }
    out_dma_eng = dma_rings[OUT_DMA]
    e_dma_eng = dma_rings[E_DMA]

    with tile.TileContext(nc) as tc, ExitStack() as ctx:
        dpool = ctx.enter_context(tc.tile_pool(name="dpool", bufs=bufs))
        epool = ctx.enter_context(tc.tile_pool(name="epool", bufs=bufs))
        zpool = ctx.enter_context(tc.tile_pool(name="zpool", bufs=bufs))
        opool = ctx.enter_context(tc.tile_pool(name="opool", bufs=bufs))
        small = ctx.enter_context(tc.tile_pool(name="small", bufs=bufs))

        def emit_front(dt_, et, o_dst, mm):
            """Stage A of one sub-tile: sum6, z = s*e, tanh, exp."""
            dv3 = dt_.rearrange("p (m c) -> p m c", c=6)
            ev = et.rearrange("p (m c) -> p m c", c=5)

            # s = sum of the 6 d components (fp16)
            s_t = small.tile([P, mm], f16, tag="s")
            with nc.allow_low_precision("fp16 pipeline, gate is 2e-2"):
                if SUM6 == "h3":
                    h3 = small.tile([P, 3 * mm], f16, tag="h3")
                    h3v = h3[:].rearrange("p (m c) -> p m c", c=3)
                    h3_eng.tensor_tensor(
                        out=h3v, in0=dv3[:, :, 0:3], in1=dv3[:, :, 3:6], op=add
                    )
                    if SUM3 == "stt":
                        p3 = small.tile([P, mm], f16, tag="p3")
                        nc.vector.scalar_tensor_tensor(
                            out=p3[:], in0=h3v[:, :, 0], scalar=1.0,
                            in1=h3v[:, :, 1], op0=mult, op1=add,
                        )
                        nc.vector.tensor_tensor(
                            out=s_t[:], in0=p3[:], in1=h3v[:, :, 2], op=add
                        )
                    else:
                        nc.vector.tensor_reduce(out=s_t[:], in_=h3v, axis=X, op=add)
                else:
                    nc.vector.tensor_reduce(out=s_t[:], in_=dv3, axis=X, op=add)

            # z = s * e (broadcast multiply, 1x)
            z = zpool.tile([P, 5 * mm], f16, tag="z")
            zv = z[:].rearrange("p (m c) -> p m c", c=5)
            s_b = s_t[:].unsqueeze(-1).broadcast_to([P, mm, 5])
            z_eng.tensor_tensor(out=zv, in0=s_b, in1=ev, op=mult)

            # t = tanh(W*z + b_c): 5 per-component calls fold scale+bias
            for c in range(5):
                nc.scalar.activation(
                    out=zv[:, :, c],
                    in_=zv[:, :, c],
                    func=ACT.Tanh,
                    bias=float(bvals[c]),
                    scale=float(W),
                )
            # a = exp(t), packed
            nc.scalar.activation(out=z[:], in_=z[:], func=ACT.Exp)
            return (z, zv, et, ev, o_dst, mm)

        def emit_back(st):
            """Stage B: softmax denom, reciprocal, w = a*e, out = w*r, store."""
            z, zv, et, ev, o_dst, mm = st
            # dnm = sum_c a (f32 out feeds the fp32-only reciprocal)
            dnm = small.tile([P, mm], f32, tag="dnm")
            if SUM5 == "tree":
                # h2 = [a0+a2, a1+a3] (packed pairs: 2x on DVE), then
                # dnm = (h2[0] + h2[1]) + a4 via two strided adds
                h2 = small.tile([P, 2 * mm], f16, tag="h2")
                h2v = h2[:].rearrange("p (m c) -> p m c", c=2)
                with nc.allow_low_precision("fp16 pipeline, gate is 2e-2"):
                    nc.vector.tensor_tensor(
                        out=h2v, in0=zv[:, :, 0:2], in1=zv[:, :, 2:4], op=add
                    )
                    p4 = small.tile([P, mm], f16, tag="p4")
                    nc.vector.scalar_tensor_tensor(
                        out=p4[:], in0=h2v[:, :, 0], scalar=1.0,
                        in1=h2v[:, :, 1], op0=mult, op1=add,
                    )
                nc.vector.tensor_tensor(
                    out=dnm[:], in0=p4[:], in1=zv[:, :, 4], op=add
                )
            else:
                nc.vector.tensor_reduce(out=dnm[:], in_=zv, axis=X, op=add)

            # r = 1/dnm (~18-bit approx; way inside the 2e-2 gate)
            r = small.tile([P, mm], f32, tag="r")
            nc.vector.reciprocal_approx_fast(out=r[:], in_=dnm[:])

            # w = a * e (packed fp16: 2x on DVE), in-place into the e tile
            w_eng.tensor_tensor(out=et, in0=z[:], in1=et, op=mult)

            # out = w * r (broadcast, 1x) into the bf16 out tile (bf16 keeps
            # the f32 exponent range, avoiding the fp16 subnormal cliff on
            # tiny outputs vs the 1e-6 rel-err guard)
            o_t = opool.tile([P, 5 * mm], bf16, tag="o")
            ov = o_t[:].rearrange("p (m c) -> p m c", c=5)
            r_b = r[:].unsqueeze(-1).broadcast_to([P, mm, 5])
            k = OUT_SPLIT_K
            if 0 < k < 5:
                nc.vector.tensor_tensor(
                    out=ov[:, :, :k], in0=ev[:, :, :k], in1=r_b[:, :, :k], op=mult
                )
                out_eng.tensor_tensor(
                    out=ov[:, :, k:], in0=ev[:, :, k:], in1=r_b[:, :, k:], op=mult
                )
            elif k >= 5:
                nc.vector.tensor_tensor(out=ov, in0=ev, in1=r_b, op=mult)
            else:
                out_eng.tensor_tensor(out=ov, in0=ev, in1=r_b, op=mult)
            if not SKIP_DMA:
                out_dma_eng.dma_start(out=o_dst, in_=o_t[:])

        pending = []  # front-emitted sub-tiles awaiting their back half

        def emit(dt_, et, o_dst, mm):
            """Compute + store one sub-tile of mm samples/partition.

            Engines run in program order, so emitting front(t) then back(t-1)
            keeps the DVE from stalling at dnm(t) waiting on the ACT
            tanh/exp round-trip of the same tile (SW_PIPE)."""
            if SKIP_COMPUTE:
                nc.sync.dma_start(out=o_dst, in_=et)
                return
            st = emit_front(dt_, et, o_dst, mm)
            pending.append(st)
            if not SW_PIPE or len(pending) > SW_DEPTH:
                emit_back(pending.pop(0))

        if SKIP_DMA:
            assert RAMP == 1 and RAMP_TAIL == 1
            for _ in range(bufs):
                dt0 = dpool.tile([P, 6 * m], f16, tag="dpool")
                nc.vector.memset(dt0[:], 0.0)
                et0 = epool.tile([P, 5 * m], f16, tag="epool")
                nc.vector.memset(et0[:], 0.0)

        def one_pass():
            for t in range(T):
                if (t == 0 and RAMP > 1) or (t == T - 1 and RAMP_TAIL > 1):
                    splits = RAMP if t == 0 else RAMP_TAIL
                    mr = m // splits
                    for k in range(splits):
                        dk = dpool.tile([P, 6 * mr], f16, tag="dpool")
                        nc.sync.dma_start(
                            out=dk[:], in_=d_v[t][:, k * 6 * mr : (k + 1) * 6 * mr]
                        )
                        ek = epool.tile([P, 5 * mr], f16, tag="epool")
                        e_dma_eng.dma_start(
                            out=ek[:], in_=e_v[t][:, k * 5 * mr : (k + 1) * 5 * mr]
                        )
                        emit(
                            dk[:], ek[:], o_v[t][:, k * 5 * mr : (k + 1) * 5 * mr], mr
                        )
                else:
                    dt_tile = dpool.tile([P, 6 * m], f16, tag="dpool")
                    et_tile = epool.tile([P, 5 * m], f16, tag="epool")
                    if not SKIP_DMA:
                        nc.sync.dma_start(out=dt_tile[:], in_=d_v[t])
                        e_dma_eng.dma_start(out=et_tile[:], in_=e_v[t])
                    emit(dt_tile[:], et_tile[:], o_v[t], m)
            while pending:  # drain the software pipeline inside the pass
                emit_back(pending.pop(0))

        rep_ctx = tc.For_i(0, repeats) if repeats > 1 else nullcontext()
        with rep_ctx:
            for _p in range(passes):
                one_pass()

    # Legalize: split multi-wait instructions (HW allows 1 wait/inst).
    nc.compile()
    return nc


def _to_f16(x):
    return np.ascontiguousarray(np.asarray(x, dtype=np.float32)).astype(np.float16)


# kept name for test.py compatibility: casts inputs to the device input dtype
_to_bf16 = _to_f16


def kernel(d, e, W, b):
    from concourse.bass_utils import run_bass_kernel_spmd

    d = _to_f16(d)
    e = _to_f16(e)
    n = d.shape[0]
    assert n % N_CORES == 0
    s = n // N_CORES

    nc = build_bass(float(np.asarray(W).reshape(-1)[0]), np.asarray(b).tolist(), s)

    in_maps = [
        {"d": d[i * s : (i + 1) * s], "e": e[i * s : (i + 1) * s]}
        for i in range(N_CORES)
    ]
    res = run_bass_kernel_spmd(nc, in_maps, list(range(N_CORES)), trace=TRACE)
    LAST["results"] = res
    out = np.concatenate([res.results[i]["out"] for i in range(N_CORES)], axis=0)
    return out.astype(np.float32)


# revision 19
# speedup vs baseline: 1.1271x; 1.1271x over previous
"""Trainium2 Bass kernel for nn_AttentionE (bf16 I/O pipeline).

Computes, per sample i:
    s_i   = sum(d_i)                       # d: (N, 6)
    z_ic  = W * s_i * e_ic + b_c           # e: (N, 5), W scalar, b: (5,)
    a_ic  = exp(tanh(z_ic))
    out_ic = e_ic * a_ic / sum_c(a_ic)     # (eps=1e-7 in ref; negligible)

Sharding: data-parallel over the sample axis across 8 NeuronCores.

This revision moves the HBM streams to bf16 (d, e inputs and the out
output), halving DMA traffic vs the f32 baseline: 64 B/sample ->
32 B/sample, memory roofline ~110us -> ~55us per pass. The pipeline
keeps f32 only where it matters (softmax denominator + reciprocal).
Host-side casts f32->bf16 (inputs) and bf16->f32 (output) keep the
kernel() contract at f32; rel-err of the bf16 path is ~1e-2 worst case
(gate 2e-2).

DVE bf16 notes (TRN2): packed 2-byte tensor_tensor runs at 2x; broadcast
APs and tensor_reduce stay 1x. So sum-of-6 is done as a packed pairwise
add (d[:, :, 0:3] + d[:, :, 3:6], 2x) + reduce3, and a*e is a packed 2x
multiply. The two broadcast multiplies (z = s*e, out = w*r) and the
reductions are the 1x DVE poles; engine assignment is env-tunable to
balance DVE against the GpSimd(Pool) engine on real HW.
"""

import sys

import numpy as np

_REPO = "/opt/trn_rl_repo"
if _REPO not in sys.path:
    sys.path.insert(0, _REPO)

from contextlib import ExitStack, nullcontext

import concourse.bacc as bacc
import concourse.bass as bass
import concourse.tile as tile
from concourse import mybir

N_CORES = 8
N_FULL = 4194304
P = 128  # SBUF partitions

import os as _os

# Tunables (env-overridable for bench sweeps)
M = int(_os.environ.get("K_M", "512"))  # samples per partition per tile
BUFS = int(_os.environ.get("K_BUFS", "4"))

# Engine assignment: "vector" or "gpsimd"
H3_ENGINE = _os.environ.get("K_H3_ENGINE", "vector")   # packed pairwise add (2x on DVE)
Z_ENGINE = _os.environ.get("K_Z_ENGINE", "gpsimd")     # z = s_b * e (broadcast, 1x)
W_ENGINE = _os.environ.get("K_W_ENGINE", "vector")     # w = a * e (packed, 2x on DVE)
OUT_ENGINE = _os.environ.get("K_OUT_ENGINE", "gpsimd") # out = w * r_b (broadcast, 1x)
# Number of out-stage components (0..5) computed on DVE instead of OUT_ENGINE.
OUT_SPLIT_K = int(_os.environ.get("K_OUT_SPLIT_K", "0"))
# sum6 mode: "h3" = packed pairwise add of the two 3-comp halves (2x on DVE),
# then SUM3 over the 3 partials; "reduce6" = plain tensor_reduce
SUM6 = _os.environ.get("K_SUM6", "h3")
# 3-way sum tail: "stt" = two strided adds (scalar_tensor_tensor + add,
# 2m elems); "reduce" = tensor_reduce over [m,3] (3m elems, 1 inst)
SUM3 = _os.environ.get("K_SUM3", "stt")
# reduce5 mode: "plain" = tensor_reduce (f16 in, f32 out, 5m elems);
# "tree" = packed h2 pairwise (2x, 2m) + strided stt-add chain (2m)
SUM5 = _os.environ.get("K_SUM5", "tree")
# Ramp-up/down: split the first/last tile into this many sub-tiles.
RAMP = int(_os.environ.get("K_RAMP", "2"))
RAMP_TAIL = int(_os.environ.get("K_RAMP_TAIL", "2"))
# Engine ring that issues the out DMA / the e-input DMA ("sync" = SP HWDGE
# ring, "scalar" = ACT HWDGE ring, "gpsimd" = SWDGE). ACT is nearly saturated
# by tanh/exp, so parking DMAs there stalls its sequencer (CoreSim): all-sync
# modeled 60.3us vs out-on-scalar 73.7us.
OUT_DMA = _os.environ.get("K_OUT_DMA", "sync")
E_DMA = _os.environ.get("K_E_DMA", "sync")
# Software-pipelined emission: emit front(t) [sum6, zmul, tanh, exp] before
# back(t-1) [sum5, recip, wmul, outmul, store] so engines (which execute in
# program order) overlap across tiles instead of stalling on the ACT
# round-trip of their own tile.
SW_PIPE = bool(int(_os.environ.get("K_SW_PIPE", "1")))
SW_DEPTH = int(_os.environ.get("K_SW_DEPTH", "1"))

# test.py can flip this to get profile/exec-time back
TRACE = False
LAST = {}

# Diagnostic modes for decomposition benches (never used by kernel()):
SKIP_COMPUTE = bool(int(_os.environ.get("K_SKIP_COMPUTE", "0")))
SKIP_DMA = bool(int(_os.environ.get("K_SKIP_DMA", "0")))


def build_bass(
    W: float,
    bvals,
    S: int,
    m: int = M,
    bufs: int = BUFS,
    repeats: int = 1,
    passes: int = 1,
):
    """Build the single-core SPMD program: d[S,6], e[S,5] bf16 -> out[S,5] bf16.

    repeats>1 wraps the whole tile loop in a hardware For_i so test.py can
    measure steady-state device time via the wall-clock slope over R.
    """
    assert S % (P * m) == 0, (S, P, m)
    T = S // (P * m)
    f32 = mybir.dt.float32
    f16 = mybir.dt.float16
    bf16 = mybir.dt.bfloat16
    mult = mybir.AluOpType.mult
    add = mybir.AluOpType.add
    X = mybir.AxisListType.X
    ACT = mybir.ActivationFunctionType

    nc = bacc.Bacc("TRN2", debug=False, num_devices=N_CORES)

    # Register the bias values as const APs so activation(bias=<float>) works.
    for i, v in enumerate(dict.fromkeys(float(x) for x in bvals)):
        t_c = nc.alloc_sbuf_tensor(f"const-bias-{i}", [P, 1], f32)
        nc.gpsimd.memset(t_c.ap(), v)
        nc.const_aps.aps[(f32, v)] = t_c.ap()
    if SKIP_COMPUTE:
        o_stat = nc.alloc_sbuf_tensor("o_stat", [P, 5 * m], bf16)
        nc.gpsimd.memset(o_stat.ap(), 0.0)
    nc.all_engine_barrier()

    d_ap = nc.dram_tensor("d", [S, 6], f16, kind="ExternalInput").ap()
    e_ap = nc.dram_tensor("e", [S, 5], f16, kind="ExternalInput").ap()
    o_ap = nc.dram_tensor("out", [S, 5], bf16, kind="ExternalOutput").ap()

    # [T, P, m*c] views; per partition the data is one contiguous DRAM run.
    d_v = d_ap.rearrange("(t p m) c -> t p (m c)", t=T, p=P, m=m)
    e_v = e_ap.rearrange("(t p m) c -> t p (m c)", t=T, p=P, m=m)
    o_v = o_ap.rearrange("(t p m) c -> t p (m c)", t=T, p=P, m=m)

    eng = {"vector": nc.vector, "gpsimd": nc.gpsimd}
    h3_eng = eng[H3_ENGINE]
    z_eng = eng[Z_ENGINE]
    w_eng = eng[W_ENGINE]
    out_eng = eng[OUT_ENGINE]
    dma_rings = {
        "sync": nc.sync,
        "scalar": nc.scalar,
        "tensor": nc.tensor,
        "gpsimd": nc.gpsimd,
    }
    out_dma_eng = dma_rings[OUT_DMA]
    e_dma_eng = dma_rings[E_DMA]

    with tile.TileContext(nc) as tc, ExitStack() as ctx:
        dpool = ctx.enter_context(tc.tile_pool(name="dpool", bufs=bufs))
        epool = ctx.enter_context(tc.tile_pool(name="epool", bufs=bufs))
        zpool = ctx.enter_context(tc.tile_pool(name="zpool", bufs=bufs))
        opool = ctx.enter_context(tc.tile_pool(name="opool", bufs=bufs))
        small = ctx.enter_context(tc.tile_pool(name="small", bufs=bufs))

        def emit_front(dt_, et, o_dst, mm):
            """Stage A of one sub-tile: sum6, z = s*e, tanh, exp."""
            dv3 = dt_.rearrange("p (m c) -> p m c", c=6)
            ev = et.rearrange("p (m c) -> p m c", c=5)

            # s = sum of the 6 d components (fp16)
            s_t = small.tile([P, mm], f16, tag="s")
            with nc.allow_low_precision("fp16 pipeline, gate is 2e-2"):
                if SUM6 == "h3":
                    h3 = small.tile([P, 3 * mm], f16, tag="h3")
                    h3v = h3[:].rearrange("p (m c) -> p m c", c=3)
                    h3_eng.tensor_tensor(
                        out=h3v, in0=dv3[:, :, 0:3], in1=dv3[:, :, 3:6], op=add
                    )
                    if SUM3 == "stt":
                        p3 = small.tile([P, mm], f16, tag="p3")
                        nc.vector.scalar_tensor_tensor(
                            out=p3[:], in0=h3v[:, :, 0], scalar=1.0,
                            in1=h3v[:, :, 1], op0=mult, op1=add,
                        )
                        nc.vector.tensor_tensor(
                            out=s_t[:], in0=p3[:], in1=h3v[:, :, 2], op=add
                        )
                    else:
                        nc.vector.tensor_reduce(out=s_t[:], in_=h3v, axis=X, op=add)
                else:
                    nc.vector.tensor_reduce(out=s_t[:], in_=dv3, axis=X, op=add)

            # z = s * e (broadcast multiply, 1x)
            z = zpool.tile([P, 5 * mm], f16, tag="z")
            zv = z[:].rearrange("p (m c) -> p m c", c=5)
            s_b = s_t[:].unsqueeze(-1).broadcast_to([P, mm, 5])
            z_eng.tensor_tensor(out=zv, in0=s_b, in1=ev, op=mult)

            # t = tanh(W*z + b_c): 5 per-component calls fold scale+bias
            for c in range(5):
                nc.scalar.activation(
                    out=zv[:, :, c],
                    in_=zv[:, :, c],
                    func=ACT.Tanh,
                    bias=float(bvals[c]),
                    scale=float(W),
                )
            # a = exp(t), packed
            nc.scalar.activation(out=z[:], in_=z[:], func=ACT.Exp)
            return (z, zv, et, ev, o_dst, mm)

        def emit_back(st):
            """Stage B: softmax denom, reciprocal, w = a*e, out = w*r, store."""
            z, zv, et, ev, o_dst, mm = st
            # dnm = sum_c a (f32 out feeds the fp32-only reciprocal)
            dnm = small.tile([P, mm], f32, tag="dnm")
            if SUM5 == "tree":
                # h2 = [a0+a2, a1+a3] (packed pairs: 2x on DVE), then
                # dnm = (h2[0] + h2[1]) + a4 via two strided adds
                h2 = small.tile([P, 2 * mm], f16, tag="h2")
                h2v = h2[:].rearrange("p (m c) -> p m c", c=2)
                with nc.allow_low_precision("fp16 pipeline, gate is 2e-2"):
                    nc.vector.tensor_tensor(
                        out=h2v, in0=zv[:, :, 0:2], in1=zv[:, :, 2:4], op=add
                    )
                    p4 = small.tile([P, mm], f16, tag="p4")
                    nc.vector.scalar_tensor_tensor(
                        out=p4[:], in0=h2v[:, :, 0], scalar=1.0,
                        in1=h2v[:, :, 1], op0=mult, op1=add,
                    )
                nc.vector.tensor_tensor(
                    out=dnm[:], in0=p4[:], in1=zv[:, :, 4], op=add
                )
            else:
                nc.vector.tensor_reduce(out=dnm[:], in_=zv, axis=X, op=add)

            # r = 1/dnm (~18-bit approx; way inside the 2e-2 gate)
            r = small.tile([P, mm], f32, tag="r")
            nc.vector.reciprocal_approx_fast(out=r[:], in_=dnm[:])

            # w = a * e (packed fp16: 2x on DVE), in-place into the e tile
            w_eng.tensor_tensor(out=et, in0=z[:], in1=et, op=mult)

            # out = w * r (broadcast, 1x) into the bf16 out tile (bf16 keeps
            # the f32 exponent range, avoiding the fp16 subnormal cliff on
            # tiny outputs vs the 1e-6 rel-err guard)
            o_t = opool.tile([P, 5 * mm], bf16, tag="o")
            ov = o_t[:].rearrange("p (m c) -> p m c", c=5)
            r_b = r[:].unsqueeze(-1).broadcast_to([P, mm, 5])
            k = OUT_SPLIT_K
            if 0 < k < 5:
                nc.vector.tensor_tensor(
                    out=ov[:, :, :k], in0=ev[:, :, :k], in1=r_b[:, :, :k], op=mult
                )
                out_eng.tensor_tensor(
                    out=ov[:, :, k:], in0=ev[:, :, k:], in1=r_b[:, :, k:], op=mult
                )
            elif k >= 5:
                nc.vector.tensor_tensor(out=ov, in0=ev, in1=r_b, op=mult)
            else:
                out_eng.tensor_tensor(out=ov, in0=ev, in1=r_b, op=mult)
            if not SKIP_DMA:
                out_dma_eng.dma_start(out=o_dst, in_=o_t[:])

        pending = []  # front-emitted sub-tiles awaiting their back half

        def emit(dt_, et, o_dst, mm):
            """Compute + store one sub-tile of mm samples/partition.

            Engines run in program order, so emitting front(t) then back(t-1)
            keeps the DVE from stalling at dnm(t) waiting on the ACT
            tanh/exp round-trip of the same tile (SW_PIPE)."""
            if SKIP_COMPUTE:
                out_dma_eng.dma_start(out=o_dst, in_=o_stat.ap()[:, : 5 * mm])
                return
            st = emit_front(dt_, et, o_dst, mm)
            pending.append(st)
            if not SW_PIPE or len(pending) > SW_DEPTH:
                emit_back(pending.pop(0))

        if SKIP_DMA:
            assert RAMP == 1 and RAMP_TAIL == 1
            for _ in range(bufs):
                dt0 = dpool.tile([P, 6 * m], f16, tag="dpool")
                nc.vector.memset(dt0[:], 0.0)
                et0 = epool.tile([P, 5 * m], f16, tag="epool")
                nc.vector.memset(et0[:], 0.0)

        def one_pass():
            for t in range(T):
                if (t == 0 and RAMP > 1) or (t == T - 1 and RAMP_TAIL > 1):
                    splits = RAMP if t == 0 else RAMP_TAIL
                    mr = m // splits
                    for k in range(splits):
                        dk = dpool.tile([P, 6 * mr], f16, tag="dpool")
                        nc.sync.dma_start(
                            out=dk[:], in_=d_v[t][:, k * 6 * mr : (k + 1) * 6 * mr]
                        )
                        ek = epool.tile([P, 5 * mr], f16, tag="epool")
                        e_dma_eng.dma_start(
                            out=ek[:], in_=e_v[t][:, k * 5 * mr : (k + 1) * 5 * mr]
                        )
                        emit(
                            dk[:], ek[:], o_v[t][:, k * 5 * mr : (k + 1) * 5 * mr], mr
                        )
                else:
                    dt_tile = dpool.tile([P, 6 * m], f16, tag="dpool")
                    et_tile = epool.tile([P, 5 * m], f16, tag="epool")
                    if not SKIP_DMA:
                        nc.sync.dma_start(out=dt_tile[:], in_=d_v[t])
                        e_dma_eng.dma_start(out=et_tile[:], in_=e_v[t])
                    emit(dt_tile[:], et_tile[:], o_v[t], m)
            while pending:  # drain the software pipeline inside the pass
                emit_back(pending.pop(0))

        rep_ctx = tc.For_i(0, repeats) if repeats > 1 else nullcontext()
        with rep_ctx:
            for _p in range(passes):
                one_pass()

    # Legalize: split multi-wait instructions (HW allows 1 wait/inst).
    nc.compile()
    return nc


def _to_f16(x):
    return np.ascontiguousarray(np.asarray(x, dtype=np.float32)).astype(np.float16)


# kept name for test.py compatibility: casts inputs to the device input dtype
_to_bf16 = _to_f16


def kernel(d, e, W, b):
    from concourse.bass_utils import run_bass_kernel_spmd

    d = _to_f16(d)
    e = _to_f16(e)
    n = d.shape[0]
    assert n % N_CORES == 0
    s = n // N_CORES

    nc = build_bass(float(np.asarray(W).reshape(-1)[0]), np.asarray(b).tolist(), s)

    in_maps = [
        {"d": d[i * s : (i + 1) * s], "e": e[i * s : (i + 1) * s]}
        for i in range(N_CORES)
    ]
    res = run_bass_kernel_spmd(nc, in_maps, list(range(N_CORES)), trace=TRACE)
    LAST["results"] = res
    out = np.concatenate([res.results[i]["out"] for i in range(N_CORES)], axis=0)
    return out.astype(np.float32)
